# revision 14
# baseline (speedup 1.0000x reference)
"""Grouped BCE-with-logits loss via segment reductions on TRN2.

Algorithm per core (data-parallel shard of N):
  g = 128*hi + lo is decomposed as lo = g & 127 (PSUM partition),
  hi = g >> 7 (PSUM column).  For each data column c (128 elements, one
  per partition), three fused DVE ops build one-hot tiles directly at
  4x rate (per-partition-scalar tensor_scalar, all operands fp16/SBUF):
    A[p, m]   = (lo_p == m)                       [128]   lhsT
    Mvc[p, j] = (hi_p == j) * (256 + v_p)         [256]   count+sum packed
    Mp[p, j]  = (hi_p == j) * t_p^256             [256]   power-max
  Two PSUM-accumulating matmuls per column:
    acc1[lo, hi] += A.T @ Mvc     (= 256*count + sum)
    acc2[lo, hi] += A.T @ Mp      (= sum of t^256)
  AllReduce (sum) across 8 cores, then decode + BCE tail:
    c = round(acc1/256), s = acc1 - 256c, m = s/c
    tmax ~= (acc2)^(1/256) via exp(ln(x)/256)  (power-max, bias ~0.4%)
    loss = (1/G) * sum(t*softplus(-m) + (1-t)*softplus(m-1))
"""
import numpy as np
from concourse import bass, bacc, mybir, tile
from concourse.bass_utils import run_bass_kernel_spmd

P = 128
G = 32768
GHI = 256
NPOW = 9          # t^(2^9) = t^512
CPACK = 256.0     # count-packing constant: moving value = CPACK + v
EPS_LHS = 512.0   # lhsT one-hot value is 2^-9 (fp8 byte 0x01); undo at tail

f32 = mybir.dt.float32
f16 = mybir.dt.float16
bf16 = mybir.dt.bfloat16
i32 = mybir.dt.int32
i16 = mybir.dt.int16
u8 = mybir.dt.uint8
f8e4 = mybir.dt.float8e4
f8e5 = mybir.dt.float8e5
Alu = mybir.AluOpType
Act = mybir.ActivationFunctionType


def build_kernel(ncores=8, F=16384, FC=512, BLK=16, collective=True,
                 dynamic=True):
    nc = bacc.Bacc("TRN2", target_bir_lowering=False, debug=False,
                   num_devices=ncores)
    inp = nc.dram_tensor("input", [P, F], f32, kind="ExternalInput")
    tgt = nc.dram_tensor("target", [P, F], f32, kind="ExternalInput")
    gid = nc.dram_tensor("gid", [P, F], i32, kind="ExternalInput")
    loss = nc.dram_tensor("loss", [1, 1], f32, kind="ExternalOutput")
    NCHUNK = F // FC
    assert NCHUNK * FC == F

    with tile.TileContext(nc) as tc:
        with tc.tile_pool(name="const", bufs=1) as constp, \
             tc.tile_pool(name="io", bufs=3) as iop, \
             tc.tile_pool(name="prep", bufs=2) as prepp, \
             tc.tile_pool(name="work", bufs=3) as workp, \
             tc.tile_pool(name="psum", bufs=1, space="PSUM") as psump, \
             tc.tile_pool(name="ptail", bufs=1, space="PSUM") as ptailp, \
             tc.tile_pool(name="tail", bufs=1) as tailp, \
             tc.tile_pool(name="dram", bufs=1, space="DRAM") as dramp:

            # ---- constants ----
            iotaA2 = constp.tile([P, P // 2], i16)   # 0..63 (lo pair index)
            nc.gpsimd.iota(iotaA2[:], pattern=[[1, P // 2]],
                           channel_multiplier=0)
            iotaB2 = constp.tile([P, GHI // 2], i16)  # 0..127 (hi pair index)
            nc.gpsimd.iota(iotaB2[:], pattern=[[1, GHI // 2]],
                           channel_multiplier=0)
            iotaB = constp.tile([P, GHI], f16)
            nc.gpsimd.iota(iotaB[:], pattern=[[1, GHI]], channel_multiplier=0,
                           allow_small_or_imprecise_dtypes=True)
            ones = constp.tile([P, 1], f32)
            nc.vector.memset(ones[:], 1.0)
            bneg1 = constp.tile([P, 1], f32)
            nc.vector.memset(bneg1[:], -1.0)

            acc1 = psump.tile([P, GHI], f32)  # 256*count + sum
            acc2 = psump.tile([P, GHI], f32)  # sum of t^256
            nc.vector.memset(acc1[:], 0.0)
            nc.vector.memset(acc2[:], 0.0)

            def chunk_iter():
                if dynamic:
                    with tc.For_i(0, F, FC,
                                  hint_engines=(mybir.EngineType.PE,)) as ci:
                        yield bass.ds(ci, FC)
                else:
                    for c in range(NCHUNK):
                        yield slice(c * FC, (c + 1) * FC)

            for sl in chunk_iter():
                vt = iop.tile([P, FC], f32, tag="vt")
                tt = iop.tile([P, FC], f32, tag="tt")
                gt = iop.tile([P, FC], i32, tag="gt")
                nc.sync.dma_start(out=vt[:], in_=inp.ap()[:, sl])
                nc.sync.dma_start(out=tt[:], in_=tgt.ap()[:, sl])
                nc.sync.dma_start(out=gt[:], in_=gid.ap()[:, sl])

                # hi = g>>7 (0..255) as f32 for the Mvc one-hot
                hi_i = prepp.tile([P, FC], i32, tag="hi_i")
                hi_f = prepp.tile([P, FC], f32, tag="hi_f")
                nc.vector.tensor_scalar(hi_i[:], gt[:], 7, None,
                                        Alu.logical_shift_right)
                nc.gpsimd.tensor_copy(hi_f[:], hi_i[:])
                vc_f = prepp.tile([P, FC], f32, tag="vc_f")
                nc.vector.tensor_scalar(vc_f[:], vt[:], CPACK, None, Alu.add)
                # lo pair index q2l = (g>>1)&63 and byte-lane scale
                # As = 1 or 256 (odd lo -> high byte of the int16 pair)
                q2l_i = prepp.tile([P, FC], i32, tag="q2l_i")
                q2l_f = prepp.tile([P, FC], f32, tag="q2l_f")
                nc.vector.tensor_scalar(q2l_i[:], gt[:], 1, 63,
                                        Alu.logical_shift_right,
                                        Alu.bitwise_and)
                nc.gpsimd.tensor_copy(q2l_f[:], q2l_i[:])
                rl_i = prepp.tile([P, FC], i32, tag="rl_i")
                rl_f = prepp.tile([P, FC], f32, tag="rl_f")
                As_f = prepp.tile([P, FC], f32, tag="As_f")
                nc.vector.tensor_scalar(rl_i[:], gt[:], 1, None,
                                        Alu.bitwise_and)
                nc.gpsimd.tensor_copy(rl_f[:], rl_i[:])
                nc.gpsimd.tensor_scalar(As_f[:], rl_f[:], 255.0, 1.0,
                                        Alu.mult, Alu.add)
                # hi pair index q2h = g>>8 and 8*parity for the byte shift
                q2h_i = prepp.tile([P, FC], i32, tag="q2h_i")
                q2h_f = prepp.tile([P, FC], f32, tag="q2h_f")
                nc.vector.tensor_scalar(q2h_i[:], gt[:], 8, None,
                                        Alu.logical_shift_right)
                nc.gpsimd.tensor_copy(q2h_f[:], q2h_i[:])
                r8h_i = prepp.tile([P, FC], i32, tag="r8h_i")
                r8h_16 = prepp.tile([P, FC], i16, tag="r8h_16")
                nc.vector.tensor_scalar(r8h_i[:], gt[:], 4, 8,
                                        Alu.logical_shift_right,
                                        Alu.bitwise_and)
                nc.gpsimd.tensor_copy(r8h_16[:], r8h_i[:])
                # tp = t^(2^NPOW) via repeated squaring on ACT
                tp_a = prepp.tile([P, FC], f32, tag="tp_a")
                tp_b = prepp.tile([P, FC], f32, tag="tp_b")
                nc.scalar.activation(tp_a[:], tt[:], Act.Square)
                cur, nxt = tp_a, tp_b
                for _ in range(NPOW - 1):
                    nc.scalar.activation(nxt[:], cur[:], Act.Square)
                    cur, nxt = nxt, cur
                # fp8e5 bits of tp, shifted into the pair byte lane, as f32
                tp8 = prepp.tile([P, FC], f8e5, tag="tp8")
                nc.vector.tensor_copy(tp8[:], cur[:])
                pb16 = prepp.tile([P, FC], i16, tag="pb16")
                nc.vector.tensor_copy(pb16[:], tp8[:].bitcast(u8))
                pbs16 = prepp.tile([P, FC], i16, tag="pbs16")
                nc.vector.tensor_tensor(pbs16[:], pb16[:], r8h_16[:],
                                        op=Alu.logical_shift_left)
                pbs_f = prepp.tile([P, FC], f32, tag="pbs_f")
                nc.vector.tensor_copy(pbs_f[:], pbs16[:])

                for cp in range(FC // 2):
                    A16p = workp.tile([P, 2, P // 2], i16, tag="A16p")
                    Mp16p = workp.tile([P, 2, GHI // 2], i16, tag="Mp16p")
                    for r in (0, 1):
                        c = 2 * cp + r
                        nc.gpsimd.tensor_scalar(A16p[:, r, :], iotaA2[:],
                                                q2l_f[:, c:c + 1],
                                                As_f[:, c:c + 1],
                                                Alu.is_equal, Alu.mult)
                        Mvc = workp.tile([P, GHI], f16, tag="Mvc")
                        if c % 4 == 3:
                            nc.gpsimd.tensor_scalar(Mvc[:], iotaB[:],
                                                    hi_f[:, c:c + 1],
                                                    vc_f[:, c:c + 1],
                                                    Alu.is_equal, Alu.mult)
                        else:
                            nc.vector.tensor_scalar(Mvc[:], iotaB[:],
                                                    hi_f[:, c:c + 1],
                                                    vc_f[:, c:c + 1],
                                                    Alu.is_equal, Alu.mult)
                        nc.vector.tensor_scalar(Mp16p[:, r, :], iotaB2[:],
                                                q2h_f[:, c:c + 1],
                                                pbs_f[:, c:c + 1],
                                                Alu.is_equal, Alu.mult)
                        nc.tensor.matmul(out=acc1[:],
                                         lhsT=A16p[:, r, :].bitcast(f8e4),
                                         rhs=Mvc[:], start=False, stop=True,
                                         skip_group_check=True)
                    nc.tensor.matmul(out=acc2[:],
                                     lhsT=A16p[:].bitcast(f8e4),
                                     rhs=Mp16p[:].bitcast(f8e5),
                                     start=False, stop=True,
                                     perf_mode=mybir.MatmulPerfMode.DoubleRow,
                                     skip_group_check=True)

            # ---- tail: allreduce + decode + BCE ----
            packed = tailp.tile([P, 2 * GHI], f32)
            nc.vector.tensor_copy(packed[:, 0:GHI], acc1[:])
            nc.vector.tensor_copy(packed[:, GHI:2 * GHI], acc2[:])
            red = tailp.tile([P, 2 * GHI], f32)
            if collective:
                ib = dramp.tile([P, 2 * GHI], f32)
                ob = dramp.tile([P, 2 * GHI], f32)
                nc.gpsimd.dma_start(ib[:], packed[:])
                nc.gpsimd.collective_compute(
                    "AllReduce", Alu.add,
                    replica_groups=[list(range(ncores))],
                    ins=[ib.opt()], outs=[ob.opt()])
                nc.sync.dma_start(red[:], ob[:])
            else:
                nc.vector.tensor_copy(red[:], packed[:])
            # undo the 2^-9 lhsT one-hot scale on both accumulators
            nc.vector.tensor_scalar_mul(red[:], red[:], EPS_LHS)
            cs = red[:, 0:GHI]          # 256*count + sum
            pw = red[:, GHI:2 * GHI]    # sum of t^(2^NPOW)
            # decode count: c = round_to_nearest(cs / 256); |sum| < 128 whp
            cq = tailp.tile([P, GHI], f32)
            nc.vector.tensor_scalar_mul(cq[:], cs, 1.0 / CPACK)
            ci_ = tailp.tile([P, GHI], i32)
            nc.vector.tensor_copy(ci_[:], cq[:])
            cf = tailp.tile([P, GHI], f32)
            nc.vector.tensor_copy(cf[:], ci_[:])
            # s = cs - 256*c
            sm = tailp.tile([P, GHI], f32)
            nc.vector.scalar_tensor_tensor(
                out=sm[:], in0=cf[:], scalar=-CPACK, in1=cs,
                op0=Alu.mult, op1=Alu.add)
            # guards (never trigger w.h.p.)
            nc.vector.tensor_scalar_max(cf[:], cf[:], 1.0)
            nc.vector.tensor_scalar_max(pw, pw, 1e-12)
            # rc = 1/count with one Newton step
            rc = tailp.tile([P, GHI], f32)
            nc.vector.reciprocal(rc[:], cf[:])
            e1 = tailp.tile([P, GHI], f32)
            nc.vector.tensor_tensor(e1[:], cf[:], rc[:], op=Alu.mult)
            nc.vector.tensor_scalar(e1[:], e1[:], 2.0, -1.0,
                                    Alu.subtract, Alu.mult)
            nc.vector.tensor_tensor(rc[:], rc[:], e1[:], op=Alu.mult)
            m = tailp.tile([P, GHI], f32)
            nc.vector.tensor_tensor(m[:], sm[:], rc[:], op=Alu.mult)
            # tmax = (sum t^256)^(1/256) = exp(ln(pw)/256)
            lnp = tailp.tile([P, GHI], f32)
            nc.scalar.activation(lnp[:], pw, Act.Ln)
            tmx = tailp.tile([P, GHI], f32)
            nc.scalar.activation(tmx[:], lnp[:], Act.Exp,
                                 scale=1.0 / (1 << NPOW))
            nc.vector.tensor_scalar_min(tmx[:], tmx[:], 1.0)
            # sp1 = softplus(-m) = ln(1 + exp(-m)), sp2 = softplus(m-1)
            sp1 = tailp.tile([P, GHI], f32)
            sp2 = tailp.tile([P, GHI], f32)
            ex = tailp.tile([P, GHI], f32)
            nc.scalar.activation(ex[:], m[:], Act.Exp, scale=-1.0)
            nc.scalar.activation(sp1[:], ex[:], Act.Ln, bias=ones[:])
            nc.scalar.activation(ex[:], m[:], Act.Exp, bias=bneg1[:])
            nc.scalar.activation(sp2[:], ex[:], Act.Ln, bias=ones[:])
            # loss_g = sp2 + tmx*(sp1-sp2)
            d = tailp.tile([P, GHI], f32)
            nc.vector.tensor_tensor(d[:], sp1[:], sp2[:], op=Alu.subtract)
            lg = tailp.tile([P, GHI], f32)
            nc.vector.tensor_tensor(lg[:], tmx[:], d[:], op=Alu.mult)
            nc.vector.tensor_tensor(lg[:], lg[:], sp2[:], op=Alu.add)
            r1 = tailp.tile([P, 1], f32)
            nc.vector.tensor_reduce(r1[:], lg[:], mybir.AxisListType.X,
                                    Alu.add)
            ps = ptailp.tile([1, 1], f32)
            nc.tensor.matmul(out=ps[:], lhsT=r1[:], rhs=ones[:],
                             start=True, stop=True, skip_group_check=True)
            sc = tailp.tile([1, 1], f32)
            nc.vector.tensor_scalar_mul(sc[:], ps[:], 1.0 / G)
            nc.sync.dma_start(loss.ap(), sc[:])

    nc.finalize()
    return nc


def run(inputs, ncores=8, F=16384, FC=512, BLK=16, nc=None):
    """inputs: dict with full arrays input/target/group_id of length ncores*P*F."""
    if nc is None:
        nc = build_kernel(ncores=ncores, F=F, FC=FC, BLK=BLK)
    n_per = P * F
    in_maps = []
    for c in range(ncores):
        sl = slice(c * n_per, (c + 1) * n_per)
        in_maps.append({
            "input": np.asarray(inputs["input"][sl], np.float32).reshape(P, F),
            "target": np.asarray(inputs["target"][sl], np.float32).reshape(P, F),
            "gid": np.asarray(inputs["group_id"][sl], np.int32).reshape(P, F),
        })
    res = run_bass_kernel_spmd(nc, in_maps, core_ids=list(range(ncores)))
    return res, float(res.results[0]["loss"][0, 0])


# ---------------------------------------------------------------------------
# Self-contained harness entry point: kernel(**inputs) -> full-shape output.
# ---------------------------------------------------------------------------
_NC_CACHE = {}


def kernel(input, target, group_id):
    ncores = 8
    n = input.shape[0]
    f = n // (ncores * P)
    assert f * ncores * P == n
    key = (ncores, f)
    if key not in _NC_CACHE:
        _NC_CACHE[key] = build_kernel(ncores=ncores, F=f)
    inputs = {"input": input, "target": target, "group_id": group_id}
    _, val = run(inputs, ncores=ncores, F=f, nc=_NC_CACHE[key])
    return np.float32(val)


# revision 34
# speedup vs baseline: 1.4510x; 1.4510x over previous
"""Grouped BCE-with-logits loss via segment reductions on TRN2.

Algorithm per core (data-parallel shard of N):
  g = 128*hi + lo is decomposed as lo = g & 127 (PSUM partition),
  hi = g >> 7 (PSUM column).  For each data column c (128 elements, one
  per partition), three fused DVE ops build one-hot tiles directly at
  4x rate (per-partition-scalar tensor_scalar, all operands fp16/SBUF):
    A[p, m]   = (lo_p == m)                       [128]   lhsT
    Mvc[p, j] = (hi_p == j) * (256 + v_p)         [256]   count+sum packed
    Mp[p, j]  = (hi_p == j) * t_p^256             [256]   power-max
  Two PSUM-accumulating matmuls per column:
    acc1[lo, hi] += A.T @ Mvc     (= 256*count + sum)
    acc2[lo, hi] += A.T @ Mp      (= sum of t^256)
  AllReduce (sum) across 8 cores, then decode + BCE tail:
    c = round(acc1/256), s = acc1 - 256c, m = s/c
    tmax ~= (acc2)^(1/256) via exp(ln(x)/256)  (power-max, bias ~0.4%)
    loss = (1/G) * sum(t*softplus(-m) + (1-t)*softplus(m-1))
"""
import numpy as np
from concourse import bass, bacc, mybir, tile
from concourse.bass_utils import run_bass_kernel_spmd

P = 128
G = 32768
GHI = 256
NPOW = 9          # t^(2^9) = t^512
CPACK = 256.0     # count-packing constant: moving value = CPACK + v
EPS_LHS = 512.0   # lhsT one-hot value is 2^-9 (fp8 byte 0x01); undo at tail

f32 = mybir.dt.float32
f16 = mybir.dt.float16
bf16 = mybir.dt.bfloat16
i32 = mybir.dt.int32
i16 = mybir.dt.int16
u8 = mybir.dt.uint8
f8e4 = mybir.dt.float8e4
f8e5 = mybir.dt.float8e5
Alu = mybir.AluOpType
Act = mybir.ActivationFunctionType


def build_kernel(ncores=8, F=16384, FC=512, BLK=16, collective=True,
                 dynamic=True, a_pool=True, mvc_pool_mod=0, mp_pool_mod=0,
                 dr=True, wbufs=4):
    nc = bacc.Bacc("TRN2", target_bir_lowering=False, debug=False,
                   num_devices=ncores)
    inp = nc.dram_tensor("input", [P, F], f32, kind="ExternalInput")
    tgt = nc.dram_tensor("target", [P, F], f32, kind="ExternalInput")
    gid = nc.dram_tensor("gid", [P, F], i32, kind="ExternalInput")
    loss = nc.dram_tensor("loss", [1, 1], f32, kind="ExternalOutput")
    NCHUNK = F // FC
    assert NCHUNK * FC == F

    with tile.TileContext(nc) as tc:
        with tc.tile_pool(name="const", bufs=1) as constp, \
             tc.tile_pool(name="io", bufs=3) as iop, \
             tc.tile_pool(name="prep", bufs=2) as prepp, \
             tc.tile_pool(name="work", bufs=wbufs) as workp, \
             tc.tile_pool(name="psum", bufs=1, space="PSUM") as psump, \
             tc.tile_pool(name="ptail", bufs=1, space="PSUM") as ptailp, \
             tc.tile_pool(name="tail", bufs=1) as tailp, \
             tc.tile_pool(name="dram", bufs=1, space="DRAM") as dramp:

            # ---- constants ----
            iotaA2 = constp.tile([P, P // 2], i16)   # 0..63 (lo pair index)
            nc.gpsimd.iota(iotaA2[:], pattern=[[1, P // 2]],
                           channel_multiplier=0)
            iotaB2 = constp.tile([P, GHI // 2], i16)  # 0..127 (hi pair index)
            nc.gpsimd.iota(iotaB2[:], pattern=[[1, GHI // 2]],
                           channel_multiplier=0)
            iotaB = constp.tile([P, GHI], f16)
            nc.gpsimd.iota(iotaB[:], pattern=[[1, GHI]], channel_multiplier=0,
                           allow_small_or_imprecise_dtypes=True)
            ones = constp.tile([P, 1], f32)
            nc.vector.memset(ones[:], 1.0)
            bneg1 = constp.tile([P, 1], f32)
            nc.vector.memset(bneg1[:], -1.0)

            acc1 = psump.tile([P, GHI], f32)  # 256*count + sum
            acc2 = psump.tile([P, GHI], f32)  # sum of t^256
            nc.vector.memset(acc1[:], 0.0)
            nc.vector.memset(acc2[:], 0.0)

            def chunk_iter():
                if dynamic:
                    with tc.For_i(0, F, FC,
                                  hint_engines=(mybir.EngineType.PE,)) as ci:
                        yield bass.ds(ci, FC)
                else:
                    for c in range(NCHUNK):
                        yield slice(c * FC, (c + 1) * FC)

            for sl in chunk_iter():
                vt = iop.tile([P, FC], f32, tag="vt")
                tt = iop.tile([P, FC], f32, tag="tt")
                gt = iop.tile([P, FC], i32, tag="gt")
                nc.sync.dma_start(out=vt[:], in_=inp.ap()[:, sl])
                nc.sync.dma_start(out=tt[:], in_=tgt.ap()[:, sl])
                nc.sync.dma_start(out=gt[:], in_=gid.ap()[:, sl])

                # hi = g>>7 (0..255) as f32 for the Mvc one-hot
                hi_i = prepp.tile([P, FC], i32, tag="hi_i")
                hi_f = prepp.tile([P, FC], f32, tag="hi_f")
                nc.vector.tensor_scalar(hi_i[:], gt[:], 7, None,
                                        Alu.logical_shift_right)
                nc.scalar.activation(hi_f[:], hi_i[:], Act.Copy)
                vc_f = prepp.tile([P, FC], f32, tag="vc_f")
                nc.scalar.activation(vc_f[:], vt[:], Act.Copy, bias=CPACK)
                # lo pair index q2l = (g>>1)&63 and byte-lane scale
                # As = 1 or 256 (odd lo -> high byte of the int16 pair)
                q2l_i = prepp.tile([P, FC], i32, tag="q2l_i")
                q2l_f = prepp.tile([P, FC], f32, tag="q2l_f")
                nc.vector.tensor_scalar(q2l_i[:], gt[:], 1, 63,
                                        Alu.logical_shift_right,
                                        Alu.bitwise_and)
                nc.scalar.activation(q2l_f[:], q2l_i[:], Act.Copy)
                rl_i = prepp.tile([P, FC], i32, tag="rl_i")
                rl_f = prepp.tile([P, FC], f32, tag="rl_f")
                As_f = prepp.tile([P, FC], f32, tag="As_f")
                nc.vector.tensor_scalar(rl_i[:], gt[:], 1, None,
                                        Alu.bitwise_and)
                nc.scalar.activation(rl_f[:], rl_i[:], Act.Copy)
                nc.scalar.activation(As_f[:], rl_f[:], Act.Identity,
                                     scale=255.0, bias=ones[:])
                # hi pair index q2h = g>>8 and 8*parity for the byte shift
                q2h_i = prepp.tile([P, FC], i32, tag="q2h_i")
                q2h_f = prepp.tile([P, FC], f32, tag="q2h_f")
                nc.vector.tensor_scalar(q2h_i[:], gt[:], 8, None,
                                        Alu.logical_shift_right)
                nc.scalar.activation(q2h_f[:], q2h_i[:], Act.Copy)
                r8h_i = prepp.tile([P, FC], i32, tag="r8h_i")
                r8h_16 = prepp.tile([P, FC], i16, tag="r8h_16")
                nc.vector.tensor_scalar(r8h_i[:], gt[:], 4, 8,
                                        Alu.logical_shift_right,
                                        Alu.bitwise_and)
                nc.vector.tensor_copy(r8h_16[:], r8h_i[:])
                # tp = t^(2^NPOW) via repeated squaring on ACT
                tp_a = prepp.tile([P, FC], f32, tag="tp_a")
                tp_b = prepp.tile([P, FC], f32, tag="tp_b")
                nc.scalar.activation(tp_a[:], tt[:], Act.Square)
                cur, nxt = tp_a, tp_b
                for _ in range(NPOW - 1):
                    nc.scalar.activation(nxt[:], cur[:], Act.Square)
                    cur, nxt = nxt, cur
                # fp8e5 bits of tp, shifted into the pair byte lane, as f32
                tp8 = prepp.tile([P, FC], f8e5, tag="tp8")
                nc.vector.tensor_copy(tp8[:], cur[:])
                pb16 = prepp.tile([P, FC], i16, tag="pb16")
                nc.vector.tensor_copy(pb16[:], tp8[:].bitcast(u8))
                pbs16 = prepp.tile([P, FC], i16, tag="pbs16")
                nc.vector.tensor_tensor(pbs16[:], pb16[:], r8h_16[:],
                                        op=Alu.logical_shift_left)
                pbs_f = prepp.tile([P, FC], f32, tag="pbs_f")
                nc.scalar.activation(pbs_f[:], pbs16[:], Act.Copy)

                if dr:
                    for cp in range(FC // 2):
                        A16p = workp.tile([P, 2, P // 2], i16, tag="A16")
                        Mp16p = workp.tile([P, 2, GHI // 2], i16, tag="Mp16")
                        for r in (0, 1):
                            c = 2 * cp + r
                            ea = nc.gpsimd if a_pool else nc.vector
                            ea.tensor_scalar(
                                A16p[:, r, :], iotaA2[:], q2l_f[:, c:c + 1],
                                As_f[:, c:c + 1], Alu.is_equal, Alu.mult)
                            Mvc = workp.tile([P, GHI], f16, tag="Mvc")
                            nc.vector.tensor_scalar(
                                Mvc[:], iotaB[:], hi_f[:, c:c + 1],
                                vc_f[:, c:c + 1], Alu.is_equal, Alu.mult)
                            nc.vector.tensor_scalar(
                                Mp16p[:, r, :], iotaB2[:], q2h_f[:, c:c + 1],
                                pbs_f[:, c:c + 1], Alu.is_equal, Alu.mult)
                            nc.tensor.matmul(
                                out=acc1[:],
                                lhsT=A16p[:, r, :].bitcast(f8e4),
                                rhs=Mvc[:], start=False, stop=True,
                                skip_group_check=True)
                        nc.tensor.matmul(
                            out=acc2[:], lhsT=A16p[:].bitcast(f8e4),
                            rhs=Mp16p[:].bitcast(f8e5),
                            start=False, stop=True,
                            perf_mode=mybir.MatmulPerfMode.DoubleRow,
                            skip_group_check=True)
                else:
                    # unpaired: per-column tiles; optional alternating Pool
                    # offload of the Mp build (own tag -> single-writer)
                    for c in range(FC):
                        mp_pool = (mp_pool_mod and
                                   c % mp_pool_mod == mp_pool_mod - 1)
                        ea = nc.gpsimd if a_pool else nc.vector
                        A16 = workp.tile([P, P // 2], i16, tag="A16")
                        ea.tensor_scalar(
                            A16[:], iotaA2[:], q2l_f[:, c:c + 1],
                            As_f[:, c:c + 1], Alu.is_equal, Alu.mult)
                        Mvc = workp.tile([P, GHI], f16, tag="Mvc")
                        nc.vector.tensor_scalar(
                            Mvc[:], iotaB[:], hi_f[:, c:c + 1],
                            vc_f[:, c:c + 1], Alu.is_equal, Alu.mult)
                        sfx = "p" if mp_pool else "d"
                        Mp16 = workp.tile([P, GHI // 2], i16, tag="Mp" + sfx)
                        (nc.gpsimd if mp_pool else nc.vector).tensor_scalar(
                            Mp16[:], iotaB2[:], q2h_f[:, c:c + 1],
                            pbs_f[:, c:c + 1], Alu.is_equal, Alu.mult)
                        nc.tensor.matmul(out=acc1[:],
                                         lhsT=A16[:].bitcast(f8e4),
                                         rhs=Mvc[:], start=False, stop=True,
                                         skip_group_check=True)
                        nc.tensor.matmul(out=acc2[:],
                                         lhsT=A16[:].bitcast(f8e4),
                                         rhs=Mp16[:].bitcast(f8e5),
                                         start=False, stop=True,
                                         skip_group_check=True)

            # ---- tail: allreduce + decode + BCE ----
            packed = tailp.tile([P, 2 * GHI], f32)
            nc.vector.tensor_copy(packed[:, 0:GHI], acc1[:])
            nc.vector.tensor_copy(packed[:, GHI:2 * GHI], acc2[:])
            red = tailp.tile([P, 2 * GHI], f32)
            if collective:
                ib = dramp.tile([P, 2 * GHI], f32)
                ob = dramp.tile([P, 2 * GHI], f32)
                nc.gpsimd.dma_start(ib[:], packed[:])
                nc.gpsimd.collective_compute(
                    "AllReduce", Alu.add,
                    replica_groups=[list(range(ncores))],
                    ins=[ib.opt()], outs=[ob.opt()])
                nc.sync.dma_start(red[:], ob[:])
            else:
                nc.vector.tensor_copy(red[:], packed[:])
            # undo the 2^-9 lhsT one-hot scale on both accumulators
            nc.vector.tensor_scalar_mul(red[:], red[:], EPS_LHS)
            cs = red[:, 0:GHI]          # 256*count + sum
            pw = red[:, GHI:2 * GHI]    # sum of t^(2^NPOW)
            # decode count: c = round_to_nearest(cs / 256); |sum| < 128 whp
            cq = tailp.tile([P, GHI], f32)
            nc.vector.tensor_scalar_mul(cq[:], cs, 1.0 / CPACK)
            ci_ = tailp.tile([P, GHI], i32)
            nc.vector.tensor_copy(ci_[:], cq[:])
            cf = tailp.tile([P, GHI], f32)
            nc.vector.tensor_copy(cf[:], ci_[:])
            # s = cs - 256*c
            sm = tailp.tile([P, GHI], f32)
            nc.vector.scalar_tensor_tensor(
                out=sm[:], in0=cf[:], scalar=-CPACK, in1=cs,
                op0=Alu.mult, op1=Alu.add)
            # guards (never trigger w.h.p.)
            nc.vector.tensor_scalar_max(cf[:], cf[:], 1.0)
            nc.vector.tensor_scalar_max(pw, pw, 1e-12)
            # rc = 1/count with one Newton step
            rc = tailp.tile([P, GHI], f32)
            nc.vector.reciprocal(rc[:], cf[:])
            e1 = tailp.tile([P, GHI], f32)
            nc.vector.tensor_tensor(e1[:], cf[:], rc[:], op=Alu.mult)
            nc.vector.tensor_scalar(e1[:], e1[:], 2.0, -1.0,
                                    Alu.subtract, Alu.mult)
            nc.vector.tensor_tensor(rc[:], rc[:], e1[:], op=Alu.mult)
            m = tailp.tile([P, GHI], f32)
            nc.vector.tensor_tensor(m[:], sm[:], rc[:], op=Alu.mult)
            # tmax = (sum t^256)^(1/256) = exp(ln(pw)/256)
            lnp = tailp.tile([P, GHI], f32)
            nc.scalar.activation(lnp[:], pw, Act.Ln)
            tmx = tailp.tile([P, GHI], f32)
            nc.scalar.activation(tmx[:], lnp[:], Act.Exp,
                                 scale=1.0 / (1 << NPOW))
            nc.vector.tensor_scalar_min(tmx[:], tmx[:], 1.0)
            # sp1 = softplus(-m) = ln(1 + exp(-m)), sp2 = softplus(m-1)
            sp1 = tailp.tile([P, GHI], f32)
            sp2 = tailp.tile([P, GHI], f32)
            ex = tailp.tile([P, GHI], f32)
            nc.scalar.activation(ex[:], m[:], Act.Exp, scale=-1.0)
            nc.scalar.activation(sp1[:], ex[:], Act.Ln, bias=ones[:])
            nc.scalar.activation(ex[:], m[:], Act.Exp, bias=bneg1[:])
            nc.scalar.activation(sp2[:], ex[:], Act.Ln, bias=ones[:])
            # loss_g = sp2 + tmx*(sp1-sp2)
            d = tailp.tile([P, GHI], f32)
            nc.vector.tensor_tensor(d[:], sp1[:], sp2[:], op=Alu.subtract)
            lg = tailp.tile([P, GHI], f32)
            nc.vector.tensor_tensor(lg[:], tmx[:], d[:], op=Alu.mult)
            nc.vector.tensor_tensor(lg[:], lg[:], sp2[:], op=Alu.add)
            r1 = tailp.tile([P, 1], f32)
            nc.vector.tensor_reduce(r1[:], lg[:], mybir.AxisListType.X,
                                    Alu.add)
            ps = ptailp.tile([1, 1], f32)
            nc.tensor.matmul(out=ps[:], lhsT=r1[:], rhs=ones[:],
                             start=True, stop=True, skip_group_check=True)
            sc = tailp.tile([1, 1], f32)
            nc.vector.tensor_scalar_mul(sc[:], ps[:], 1.0 / G)
            nc.sync.dma_start(loss.ap(), sc[:])

    nc.finalize()
    return nc


def run(inputs, ncores=8, F=16384, FC=512, BLK=16, nc=None):
    """inputs: dict with full arrays input/target/group_id of length ncores*P*F."""
    if nc is None:
        nc = build_kernel(ncores=ncores, F=F, FC=FC, BLK=BLK)
    n_per = P * F
    in_maps = []
    for c in range(ncores):
        sl = slice(c * n_per, (c + 1) * n_per)
        in_maps.append({
            "input": np.asarray(inputs["input"][sl], np.float32).reshape(P, F),
            "target": np.asarray(inputs["target"][sl], np.float32).reshape(P, F),
            "gid": np.asarray(inputs["group_id"][sl], np.int32).reshape(P, F),
        })
    res = run_bass_kernel_spmd(nc, in_maps, core_ids=list(range(ncores)))
    return res, float(res.results[0]["loss"][0, 0])


# ---------------------------------------------------------------------------
# Self-contained harness entry point: kernel(**inputs) -> full-shape output.
# ---------------------------------------------------------------------------
_NC_CACHE = {}


def kernel(input, target, group_id):
    ncores = 8
    n = input.shape[0]
    f = n // (ncores * P)
    assert f * ncores * P == n
    key = (ncores, f)
    if key not in _NC_CACHE:
        _NC_CACHE[key] = build_kernel(ncores=ncores, F=f)
    inputs = {"input": input, "target": target, "group_id": group_id}
    _, val = run(inputs, ncores=ncores, F=f, nc=_NC_CACHE[key])
    return np.float32(val)
